# revision 47
# baseline (speedup 1.0000x reference)
"""GCN graph classifier on 8 Trainium2 NeuronCores (Bass/Tile).

Graphs (and their nodes) are sharded across the 8 cores; each layer's
node table x*dinv@W is replicated via AllGather in bf16.  Message
aggregation = indirect-DMA gathers (degree-class packed, one index
column per instruction — the HW's random-gather granularity) +
block-ones PE matmuls that sum each node's messages on the tensor
engine.  Layer-1 messages are pre-gathered on the host (indices are
host-known), so layer 1 streams bulk DMA instead of gathers.
GraphNorm runs as one fused pass: y is bounced to DRAM, per-graph
sums come from one gather set + ones-vector matmuls (y^2 is squared
in SBUF from the same gathered values), all scalar math happens in
graph space, and a single indirect gather broadcasts the fused affine
(A, B) back to the nodes: out = y*A[g] + B[g].
"""
import numpy as np
import ml_dtypes

import concourse.bacc as bacc
import concourse.bass as bass
import concourse.mybir as mybir
import concourse.tile as tile
from concourse.bass_utils import run_bass_kernel_spmd

BF = ml_dtypes.bfloat16

N_NODES = 100000
N_GRAPHS = 1024
HID = 32
EPS = 1e-5
NC = 8
P = 128
GPC = N_GRAPHS // NC          # graphs per core
KP_CLASSES = [(12, 10), (14, 9), (16, 8), (18, 7), (21, 6), (25, 5),
              (32, 4), (42, 3), (64, 2), (128, 1)]
MM_COLS = 16                  # idx-cols (node column-groups) per matmul
GCH = 160                     # idx-cols per gather chunk
LANES = (0, 32, 64)
FG_BANKS = 2                  # psum banks per flush group
LG_MM = FG_BANKS              # matmuls per lane-group
f32 = mybir.dt.float32
bf16 = mybir.dt.bfloat16
i32 = mybir.dt.int32

SIM_INIT_PSUM = False
DEBUG = False


def _preprocess_structure(edge_index, batch):
    ei = np.asarray(edge_index, dtype=np.int64)
    row = np.concatenate([ei[0], np.arange(N_NODES, dtype=np.int64)])
    col = np.concatenate([ei[1], np.arange(N_NODES, dtype=np.int64)])
    batch = np.asarray(batch, dtype=np.int64)

    deg = np.bincount(col, minlength=N_NODES)
    assert deg.max() <= 128, f"max degree {deg.max()} > 128"
    dinv = (1.0 / np.sqrt(np.maximum(deg, 1.0))).astype(np.float32)
    cnt = np.bincount(batch, minlength=N_GRAPHS)
    assert cnt.max() <= 256, f"max graph size {cnt.max()} > 256"
    inv_cnt = (1.0 / np.maximum(cnt, 1.0)).astype(np.float32)

    order = np.argsort(col, kind="stable")
    srcs = row[order]
    indptr = np.zeros(N_NODES + 1, np.int64)
    np.cumsum(np.bincount(col, minlength=N_NODES), out=indptr[1:])

    node_core = (batch // GPC).astype(np.int64)
    core_start = np.searchsorted(batch, np.arange(0, N_GRAPHS + 1, GPC))

    kp_arr = np.array([k for k, _ in KP_CLASSES])
    cls_of = np.searchsorted(kp_arr, deg)

    members = []
    for c in range(NC):
        lo, hi = core_start[c], core_start[c + 1]
        ids = np.arange(lo, hi)
        members.append([ids[cls_of[lo:hi] == k] for k in range(len(KP_CLASSES))])

    cols_k = []
    for k, (kp, npc) in enumerate(KP_CLASSES):
        m = max((len(members[c][k]) + npc - 1) // npc for c in range(NC))
        cols_k.append(-(-m // MM_COLS) * MM_COLS if m else 0)

    mms = []
    for k, (kp, npc) in enumerate(KP_CLASSES):
        for a in range(0, cols_k[k], MM_COLS):
            mms.append((k, a, min(MM_COLS, cols_k[k] - a)))
    n_mm = len(mms)

    descs = []
    R = 0
    B = 0
    for m0 in range(0, n_mm, LG_MM):
        grp = mms[m0:m0 + LG_MM]
        npc_max = max(KP_CLASSES[k][1] for k, _, _ in grp)
        lg = m0 // LG_MM
        lane = LANES[lg % 3]
        fg = lg // 3
        if R + npc_max > P:
            R = 0
            B += FG_BANKS * MM_COLS
        for i, (k, a, ncols) in enumerate(grp):
            descs.append(dict(k=k, a=a, ncols=ncols, npc=KP_CLASSES[k][1],
                              fg=fg, lane=lane, bank=i,
                              aggR=R, band=B + i * MM_COLS))
        R += npc_max
    NX = B + FG_BANKS * MM_COLS
    NX = -(-NX // 4) * 4
    SH = P * NX
    TBL = NC * SH
    CW = sum(cols_k)
    CWP = -(-CW // GCH) * GCH

    agg_p = np.full(N_NODES, -1, np.int32)
    agg_x = np.full(N_NODES, -1, np.int32)
    col_base = np.concatenate([[0], np.cumsum(cols_k)[:-1]]).astype(np.int64)
    for c in range(NC):
        for d in descs:
            k, a, ncols, npc = d["k"], d["a"], d["ncols"], d["npc"]
            mem = members[c][k]
            for j in range(ncols):
                nodes = mem[(a + j) * npc:(a + j + 1) * npc]
                agg_p[nodes] = d["aggR"] + np.arange(len(nodes))
                agg_x[nodes] = d["band"] + j
    r_local = agg_p.astype(np.int64) * NX + agg_x
    r_global = node_core * SH + r_local

    idx_msgs = np.full((NC, P, CWP), TBL, np.int32)
    for c in range(NC):
        for d in descs:
            k, a, ncols, npc = d["k"], d["a"], d["ncols"], d["npc"]
            kp = KP_CLASSES[k][0]
            mem = members[c][k]
            gc0 = col_base[k] + a
            for j in range(ncols):
                nodes = mem[(a + j) * npc:(a + j + 1) * npc]
                for l, v in enumerate(nodes):
                    dv = deg[v]
                    idx_msgs[c, l * kp:l * kp + dv, gc0 + j] = \
                        r_global[srcs[indptr[v]:indptr[v + 1]]]

    msg_col_real = (idx_msgs != TBL).any(axis=(0, 1))        # [CWP]

    # two 128-slot columns per graph (graph sizes can exceed 128)
    idx_stats = np.full((NC, P, 2 * GPC), SH, np.int32)
    for c in range(NC):
        lo, hi = core_start[c], core_start[c + 1]
        b_loc = batch[lo:hi] - c * GPC
        ids = np.arange(lo, hi)
        for g in range(GPC):
            ms = ids[b_loc == g]
            n0 = min(len(ms), P)
            idx_stats[c, :n0, 2 * g] = r_local[ms[:n0]]
            if len(ms) > P:
                idx_stats[c, :len(ms) - P, 2 * g + 1] = r_local[ms[P:]]
    stat_col_real = (idx_stats != SH).any(axis=(0, 1))       # [2*GPC]

    idx_bc = np.full((NC, P, NX), GPC, np.int32)
    dinv_agg = np.zeros((NC, P, NX), np.float32)
    for c in range(NC):
        ids = np.arange(core_start[c], core_start[c + 1])
        idx_bc[c, agg_p[ids], agg_x[ids]] = (batch[ids] - c * GPC).astype(np.int32)
        dinv_agg[c, agg_p[ids], agg_x[ids]] = dinv[ids]

    invcnt_col = inv_cnt.reshape(NC, GPC)

    ones_all = np.zeros((P, sum(n for _, n in KP_CLASSES)), BF)
    ones_off = []
    off = 0
    for kp, npc in KP_CLASSES:
        ones_off.append(off)
        for l in range(npc):
            ones_all[l * kp:(l + 1) * kp, off + l] = 1
        off += npc

    return dict(
        deg=deg, dinv=dinv, inv_cnt=inv_cnt,
        descs=descs, cols_k=cols_k, col_base=col_base, NX=NX, SH=SH,
        TBL=TBL, CW=CW, CWP=CWP,
        r_global=r_global, r_local=r_local, agg_p=agg_p, agg_x=agg_x,
        core_start=core_start, msg_col_real=msg_col_real,
        stat_col_real=stat_col_real,
        idx_msgs=idx_msgs, idx_stats=idx_stats, idx_bc=idx_bc,
        dinv_agg=dinv_agg, invcnt_col=invcnt_col,
        ones_all=ones_all, ones_off=ones_off,
    )


def _build(plan):
    NX, SH, TBL, CWP = plan["NX"], plan["SH"], plan["TBL"], plan["CWP"]
    NXF = NX * HID
    NB = NX // 4
    NONES = plan["ones_all"].shape[1]
    NST = 2 * GPC * HID // 512    # stats matmuls per source (=16)
    msg_col_real = plan["msg_col_real"]
    stat_col_real = plan["stat_col_real"]

    nc_ = bacc.Bacc(None, target_bir_lowering=False)

    mg = nc_.declare_dram_parameter("mg", [P, CWP * HID], bf16, isOutput=False)
    idxm = nc_.declare_dram_parameter("idxm", [P, CWP], i32, isOutput=False)
    idxs = nc_.declare_dram_parameter("idxs", [P, 2 * GPC], i32, isOutput=False)
    idxb = nc_.declare_dram_parameter("idxb", [P, NX], i32, isOutput=False)
    dinv_in = nc_.declare_dram_parameter("dinv", [P, NX], f32, isOutput=False)
    invc_in = nc_.declare_dram_parameter("invc", [P, 1], f32, isOutput=False)
    ones_in = nc_.declare_dram_parameter("ones", [P, NONES], bf16, isOutput=False)
    prm_in = nc_.declare_dram_parameter("prm", [16, HID], f32, isOutput=False)
    pidx_in = nc_.declare_dram_parameter("pidx", [P, 16], i32, isOutput=False)
    w4_2_in = nc_.declare_dram_parameter("w4_2", [P, P], bf16, isOutput=False)
    w4_3_in = nc_.declare_dram_parameter("w4_3", [P, P], bf16, isOutput=False)
    wl_in = nc_.declare_dram_parameter("wl", [HID, 3], f32, isOutput=False)
    idb_in = nc_.declare_dram_parameter("idb", [P, P], bf16, isOutput=False)
    idf_in = nc_.declare_dram_parameter("idf", [P, P], f32, isOutput=False)
    out_t = nc_.declare_dram_parameter("out", [N_GRAPHS, 3], f32, isOutput=True)

    tabA = nc_.dram_tensor("tabA", [TBL + 1, HID], bf16)
    tabB = nc_.dram_tensor("tabB", [TBL + 1, HID], bf16)
    stg = nc_.dram_tensor("stg", [SH, HID], bf16)
    xb1 = nc_.dram_tensor("xb1", [SH + 1, HID], bf16)
    cbd = nc_.dram_tensor("cbd", [GPC + 1, 2 * HID], f32)
    srd = nc_.dram_tensor("srd", [4 * GPC, HID], f32)
    lg_in = nc_.dram_tensor("lg_in", [GPC, 3], f32)
    lg_out = nc_.dram_tensor("lg_out", [N_GRAPHS, 3], f32)

    RG = [list(range(NC))]
    AX = mybir.AluOpType
    ACT = mybir.ActivationFunctionType

    with tile.TileContext(nc_) as tc:
        with (
            tc.tile_pool(name="persist", bufs=1) as pp,
            tc.tile_pool(name="work", bufs=1) as wp,
            tc.tile_pool(name="gather", bufs=2) as gp,
            tc.tile_pool(name="stat", bufs=1) as sp,
        ):
            idxm_t = pp.tile([P, CWP], i32)
            nc_.sync.dma_start(out=idxm_t[:], in_=idxm[:, :])
            idxs_t = pp.tile([P, 2 * GPC], i32)
            nc_.sync.dma_start(out=idxs_t[:], in_=idxs[:, :])
            idxb_t = pp.tile([P, NX], i32)
            nc_.sync.dma_start(out=idxb_t[:], in_=idxb[:, :])
            dinv_t = pp.tile([P, NX], f32)
            nc_.sync.dma_start(out=dinv_t[:], in_=dinv_in[:, :])
            dinv_b = pp.tile([P, NX], bf16)
            nc_.vector.tensor_copy(out=dinv_b[:], in_=dinv_t[:])
            invc_t = pp.tile([P, 1], f32)
            nc_.sync.dma_start(out=invc_t[:], in_=invc_in[:, :])
            ones_t = pp.tile([P, NONES], bf16)
            nc_.sync.dma_start(out=ones_t[:], in_=ones_in[:, :])
            w42_t = pp.tile([P, P], bf16)
            nc_.sync.dma_start(out=w42_t[:], in_=w4_2_in[:, :])
            w43_t = pp.tile([P, P], bf16)
            nc_.sync.dma_start(out=w43_t[:], in_=w4_3_in[:, :])
            wl_t = pp.tile([HID, 3], f32)
            nc_.sync.dma_start(out=wl_t[:], in_=wl_in[:, :])
            idb_t = pp.tile([P, P], bf16)
            nc_.sync.dma_start(out=idb_t[:], in_=idb_in[:, :])
            idf_t = pp.tile([P, P], f32)
            nc_.sync.dma_start(out=idf_t[:], in_=idf_in[:, :])
            pidx_t = pp.tile([P, 16], i32)
            nc_.sync.dma_start(out=pidx_t[:], in_=pidx_in[:, :])
            prm_t = pp.tile([P, 16 * HID], f32)
            for jj in range(16):
                nc_.gpsimd.indirect_dma_start(
                    out=prm_t[:, jj * HID:(jj + 1) * HID],
                    out_offset=None, in_=prm_in[:],
                    in_offset=bass.IndirectOffsetOnAxis(
                        ap=pidx_t[:, jj:jj + 1], axis=0))
            ocol_t = pp.tile([P, 1], bf16)
            nc_.vector.memset(ocol_t[:], 1.0)

            zrow = pp.tile([1, 2 * HID], f32)
            nc_.vector.memset(zrow[:], 0.0)
            zrow_b = pp.tile([1, HID], bf16)
            nc_.vector.memset(zrow_b[:], 0.0)
            nc_.sync.dma_start(out=tabA[TBL:TBL + 1, :], in_=zrow_b[:])
            nc_.sync.dma_start(out=tabB[TBL:TBL + 1, :], in_=zrow_b[:])
            nc_.sync.dma_start(out=xb1[SH:SH + 1, :], in_=zrow_b[:])
            nc_.sync.dma_start(out=cbd[GPC:GPC + 1, :], in_=zrow[:])

            def prm_row(r):
                return prm_t[:, r * HID:(r + 1) * HID]

            def as3(ap):
                return ap.rearrange("p (x f) -> p x f", f=HID)

            def stats_sums(bounce, tag):
                """bounce [SH+1] bf16 -> (s1, s2) per-graph sums of y and
                y^2, [GPC, HID] f32 each.  One gather set (real columns
                only, padded columns zeroed by memset); y^2 squared in
                SBUF from the gathered values."""
                st = sp.tile([P, 2 * GPC * HID], bf16, tag="stbuf")
                nc_.vector.memset(st[:], 0.0)
                for jj in range(2 * GPC):
                    if stat_col_real[jj]:
                        nc_.gpsimd.indirect_dma_start(
                            out=st[:, jj * HID:(jj + 1) * HID],
                            out_offset=None, in_=bounce[:],
                            in_offset=bass.IndirectOffsetOnAxis(
                                ap=idxs_t[:, jj:jj + 1], axis=0))
                stq = sp.tile([P, 2 * GPC * HID], bf16, tag="stq")
                nc_.vector.tensor_tensor(out=stq[:], in0=st[:], in1=st[:],
                                         op=AX.mult)

                def half(src, blk):
                    srow = wp.tile([1, NST * 512], f32, tag="srow")
                    with tc.tile_pool(name=f"psS{tag}{blk}", bufs=2,
                                      space="PSUM") as psS:
                        for m in range(NST):
                            pss = psS.tile([1, 512], f32, space="PSUM",
                                           tag="sps")
                            nc_.tensor.matmul(
                                out=pss[:], lhsT=ocol_t[:],
                                rhs=src[:, m * 512:(m + 1) * 512],
                                start=True, stop=True)
                            dst = srow[:, m * 512:(m + 1) * 512]
                            if m % 2 == 0:
                                nc_.scalar.activation(out=dst, in_=pss[:],
                                                      func=ACT.Copy)
                            else:
                                nc_.vector.tensor_copy(out=dst, in_=pss[:])
                    nc_.sync.dma_start(
                        out=srd[blk * 2 * GPC:(blk + 1) * 2 * GPC, :],
                        in_=srow[:])
                    sw = wp.tile([GPC, 2 * HID], f32, tag=f"sw{blk}")
                    nc_.sync.dma_start(
                        out=sw[:],
                        in_=srd[blk * 2 * GPC:(blk + 1) * 2 * GPC, :]
                        .rearrange("(g two) f -> g (two f)", two=2))
                    s = wp.tile([GPC, HID], f32, tag=tag + f"s{blk}")
                    nc_.vector.tensor_tensor(out=s[:], in0=sw[:, 0:HID],
                                             in1=sw[:, HID:2 * HID],
                                             op=AX.add)
                    return s

                s1 = half(st, 0)
                s2 = half(stq, 1)
                return s1, s2

            descs = plan["descs"]
            col_base = plan["col_base"]
            ones_off = plan["ones_off"]
            tables = [None, tabA, tabB]
            x_prev = {}

            for L in range(3):
                agg = wp.tile([P, NXF], f32, tag="agg")
                nc_.vector.memset(agg[:], 0.0)
                # ---------- aggregation ----------
                gbufs = {}
                with tc.tile_pool(name=f"psA{L}", bufs=2, space="PSUM") as psA:
                    ps = None
                    fg_list = []
                    last_fg = -1

                    def flush(ps, fg_list):
                        sc = wp.tile([P, FG_BANKS * 512], f32, tag="scr")
                        nc_.scalar.activation(out=sc[:], in_=ps[:],
                                              func=ACT.Copy)
                        for dd in fg_list:
                            nc_.sync.dma_start(
                                out=agg[dd["aggR"]:dd["aggR"] + dd["npc"],
                                        dd["band"] * HID:
                                        (dd["band"] + dd["ncols"]) * HID],
                                in_=sc[dd["lane"]:dd["lane"] + dd["npc"],
                                       dd["bank"] * 512:
                                       dd["bank"] * 512 + dd["ncols"] * HID])

                    for m, d in enumerate(descs):
                        gcol = int(col_base[d["k"]] + d["a"])
                        ch = gcol // GCH
                        if ch not in gbufs:
                            gb = gp.tile([P, GCH * HID], bf16, tag="gbuf")
                            if L == 0:
                                nc_.sync.dma_start(
                                    out=gb[:],
                                    in_=mg[:, ch * GCH * HID:
                                           (ch + 1) * GCH * HID])
                            else:
                                for j in range(GCH):
                                    jc = ch * GCH + j
                                    if jc < CWP and msg_col_real[jc]:
                                        nc_.gpsimd.indirect_dma_start(
                                            out=gb[:, j * HID:(j + 1) * HID],
                                            out_offset=None,
                                            in_=tables[L][:],
                                            in_offset=bass.IndirectOffsetOnAxis(
                                                ap=idxm_t[:, jc:jc + 1],
                                                axis=0))
                            gbufs[ch] = gb
                        if d["fg"] != last_fg:
                            if ps is not None:
                                flush(ps, fg_list)
                            ps = psA.tile([P, FG_BANKS * 512], f32,
                                          space="PSUM", tag="aggps")
                            if SIM_INIT_PSUM:
                                nc_.vector.memset(ps[:], 0.0)
                            fg_list = []
                            last_fg = d["fg"]
                        loc = gcol - ch * GCH
                        off = ones_off[d["k"]]
                        nc_.tensor.matmul(
                            out=ps[d["lane"]:d["lane"] + d["npc"],
                                   d["bank"] * 512:
                                   d["bank"] * 512 + d["ncols"] * HID],
                            lhsT=ones_t[:, off:off + d["npc"]],
                            rhs=gbufs[ch][:, loc * HID:(loc + d["ncols"]) * HID],
                            start=True, stop=True)
                        fg_list.append(d)
                    flush(ps, fg_list)

                # ---------- y = agg * dinv ----------
                nc_.vector.tensor_tensor(
                    out=as3(agg[:]), in0=as3(agg[:]),
                    in1=dinv_t[:][:, :, None].to_broadcast([P, NX, HID]),
                    op=AX.mult)

                # ---------- graph norm (fused single pass) ----------
                nc_.gpsimd.dma_start(   # cast f32->bf16 in flight
                    out=xb1[0:SH, :].rearrange("(p x) f -> p (x f)", p=P),
                    in_=agg[:])
                s1, s2 = stats_sums(xb1, f"st{L}")

                # graph-space math: x = y + b;  m = E[x]; e2 = E[x^2]
                my = wp.tile([GPC, HID], f32, tag="my")
                nc_.vector.tensor_scalar_mul(out=my[:], in0=s1[:],
                                             scalar1=invc_t[:, 0:1])
                mm = wp.tile([GPC, HID], f32, tag="mm")
                nc_.vector.tensor_tensor(out=mm[:], in0=my[:],
                                         in1=prm_row(0 + L), op=AX.add)
                e2 = wp.tile([GPC, HID], f32, tag="e2")
                nc_.vector.tensor_scalar(out=e2[:], in0=s2[:],
                                         scalar1=invc_t[:, 0:1], scalar2=EPS,
                                         op0=AX.mult, op1=AX.add)
                u = wp.tile([GPC, HID], f32, tag="u")
                nc_.vector.tensor_tensor(out=u[:], in0=my[:], in1=mm[:],
                                         op=AX.add)
                nc_.vector.tensor_tensor(out=u[:], in0=u[:],
                                         in1=prm_row(0 + L), op=AX.mult)
                nc_.vector.tensor_tensor(out=e2[:], in0=e2[:], in1=u[:],
                                         op=AX.add)
                msm = wp.tile([GPC, HID], f32, tag="msm")
                nc_.vector.tensor_tensor(out=msm[:], in0=mm[:],
                                         in1=prm_row(9 + L), op=AX.mult)
                r = wp.tile([GPC, HID], f32, tag="r")
                nc_.vector.tensor_tensor(out=r[:], in0=mm[:], in1=msm[:],
                                         op=AX.subtract)
                nc_.vector.tensor_tensor(out=r[:], in0=r[:], in1=mm[:],
                                         op=AX.add)
                nc_.vector.tensor_tensor(out=r[:], in0=r[:], in1=msm[:],
                                         op=AX.mult)
                var = wp.tile([GPC, HID], f32, tag="var")
                nc_.vector.tensor_tensor(out=var[:], in0=e2[:], in1=r[:],
                                         op=AX.subtract)
                rstd = wp.tile([GPC, HID], f32, tag="rstd")
                nc_.vector.reciprocal(out=rstd[:], in_=var[:])
                nc_.scalar.activation(out=rstd[:], in_=rstd[:], func=ACT.Sqrt)
                ab = wp.tile([GPC, 2 * HID], f32, tag="ab")
                nc_.vector.tensor_tensor(out=ab[:, 0:HID], in0=rstd[:],
                                         in1=prm_row(6 + L), op=AX.mult)
                d2 = wp.tile([GPC, HID], f32, tag="d2")
                nc_.vector.tensor_tensor(out=d2[:], in0=prm_row(0 + L),
                                         in1=msm[:], op=AX.subtract)
                nc_.vector.tensor_tensor(out=d2[:], in0=d2[:],
                                         in1=ab[:, 0:HID], op=AX.mult)
                nc_.vector.tensor_tensor(out=ab[:, HID:2 * HID], in0=d2[:],
                                         in1=prm_row(3 + L), op=AX.add)
                # broadcast AB to node slots (256B row per graph)
                nc_.sync.dma_start(out=cbd[0:GPC, :], in_=ab[:])
                outb = wp.tile([P, NX * 2 * HID], f32, tag="bcbuf")
                for x in range(NX):
                    nc_.gpsimd.indirect_dma_start(
                        out=outb[:, x * 2 * HID:(x + 1) * 2 * HID],
                        out_offset=None, in_=cbd[:],
                        in_offset=bass.IndirectOffsetOnAxis(
                            ap=idxb_t[:, x:x + 1], axis=0))
                obv = outb[:].rearrange("p (x tf) -> p x tf", tf=2 * HID)
                # fused: xl = relu(y*A + B (+ x_prev))
                nc_.vector.tensor_tensor(out=as3(agg[:]), in0=as3(agg[:]),
                                         in1=obv[:, :, 0:HID], op=AX.mult)
                nc_.vector.tensor_tensor(out=as3(agg[:]), in0=as3(agg[:]),
                                         in1=obv[:, :, HID:2 * HID],
                                         op=AX.add)
                xl = wp.tile([P, NXF], bf16, tag=f"x{L % 2}")
                if L > 0:
                    nc_.vector.tensor_copy(out=xl[:], in_=agg[:])
                    nc_.vector.tensor_tensor(out=xl[:], in0=xl[:],
                                             in1=x_prev[L - 1][:], op=AX.add)
                    nc_.scalar.activation(out=xl[:], in_=xl[:], func=ACT.Relu)
                else:
                    nc_.scalar.activation(out=xl[:], in_=agg[:], func=ACT.Relu)
                x_prev[L] = xl

                if L < 2:
                    # ---------- next table ----------
                    w4 = w42_t if L == 0 else w43_t
                    tab = tabA if L == 0 else tabB
                    yb = wp.tile([P, NXF], bf16, tag="yb")
                    nc_.vector.tensor_tensor(
                        out=as3(yb[:]), in0=as3(xl[:]),
                        in1=dinv_b[:][:, :, None].to_broadcast([P, NX, HID]),
                        op=AX.mult)
                    xt4 = wp.tile([P, NXF], bf16, tag="xt4")
                    with tc.tile_pool(name=f"psT{L}", bufs=2, space="PSUM") as psT:
                        for b in range(NB):
                            pst = psT.tile([P, P], bf16, space="PSUM", tag="tp")
                            nc_.tensor.transpose(out=pst[:],
                                                 in_=yb[:, b * P:(b + 1) * P],
                                                 identity=idb_t[:])
                            nc_.vector.tensor_copy(
                                out=xt4[:, b * P:(b + 1) * P], in_=pst[:])
                    zt4 = wp.tile([P, NXF], bf16, tag="zt4")
                    with tc.tile_pool(name=f"psW{L}", bufs=2, space="PSUM") as psW:
                        for q in range(NXF // 512):
                            psw = psW.tile([P, 512], f32, space="PSUM", tag="wp")
                            nc_.tensor.matmul(out=psw[:], lhsT=w4[:],
                                              rhs=xt4[:, q * 512:(q + 1) * 512],
                                              start=True, stop=True)
                            nc_.vector.tensor_copy(
                                out=zt4[:, q * 512:(q + 1) * 512], in_=psw[:])
                    zb = wp.tile([P, NXF], bf16, tag="zb")
                    with tc.tile_pool(name=f"psU{L}", bufs=2, space="PSUM") as psU:
                        for b in range(NB):
                            pst = psU.tile([P, P], bf16, space="PSUM", tag="tp2")
                            nc_.tensor.transpose(out=pst[:],
                                                 in_=zt4[:, b * P:(b + 1) * P],
                                                 identity=idb_t[:])
                            nc_.scalar.activation(
                                out=zb[:, b * P:(b + 1) * P], in_=pst[:],
                                func=ACT.Copy)
                    nc_.sync.dma_start(
                        out=stg[:, :].rearrange("(p x) f -> p (x f)", p=P),
                        in_=zb[:])
                    nc_.gpsimd.collective_compute(
                        "AllGather", AX.bypass, replica_groups=RG,
                        ins=[stg[:, :]], outs=[tab[0:TBL, :]])
                else:
                    # ---------- pool + logits ----------
                    nc_.sync.dma_start(
                        out=xb1[0:SH, :].rearrange("(p x) f -> p (x f)", p=P),
                        in_=xl[:])
                    s3, _s3q = stats_sums(xb1, "stF")
                    pooled = wp.tile([GPC, HID], f32, tag="pooled")
                    nc_.vector.tensor_scalar_mul(out=pooled[:], in0=s3[:],
                                                 scalar1=invc_t[:, 0:1])
                    with tc.tile_pool(name="psF", bufs=1, space="PSUM") as psF:
                        pstp = psF.tile([HID, P], f32, space="PSUM", tag="pt")
                        nc_.tensor.transpose(out=pstp[:], in_=pooled[:],
                                             identity=idf_t[:])
                        pooledT = wp.tile([HID, P], f32, tag="pooledT")
                        nc_.vector.tensor_copy(out=pooledT[:], in_=pstp[:])
                        psl = psF.tile([P, 3], f32, space="PSUM", tag="lg")
                        nc_.tensor.matmul(out=psl[:], lhsT=pooledT[:],
                                          rhs=wl_t[:], start=True, stop=True)
                        logits = wp.tile([GPC, 3], f32, tag="logits")
                        nc_.vector.tensor_tensor(
                            out=logits[:], in0=psl[:],
                            in1=prm_t[0:GPC, 12 * HID:12 * HID + 3], op=AX.add)
                    nc_.sync.dma_start(out=lg_in[:, :], in_=logits[:])
                    nc_.gpsimd.collective_compute(
                        "AllGather", AX.bypass, replica_groups=RG,
                        ins=[lg_in[:, :]], outs=[lg_out[:, :]])
                    ofin = wp.tile([P, N_GRAPHS * 3 // P], f32, tag="ofin")
                    nc_.sync.dma_start(
                        out=ofin[:],
                        in_=lg_out[:, :].rearrange("(p a) f -> p (a f)", p=P))
                    nc_.sync.dma_start(
                        out=out_t[:, :].rearrange("(p a) f -> p (a f)", p=P),
                        in_=ofin[:])

    nc_.finalize()
    return nc_


_CACHE = {}


def _get_plan_nc(edge_index, batch):
    key = (hash(np.asarray(edge_index)[:, ::997].tobytes()),
           hash(np.asarray(batch)[::97].tobytes()))
    if key not in _CACHE:
        plan = _preprocess_structure(edge_index, batch)
        nc_ = _build(plan)
        _CACHE[key] = (plan, nc_)
    return _CACHE[key]


def kernel(x, edge_index, batch, W1, b1, W2, b2, W3, b3,
           g1, be1, ms1, g2, be2, ms2, g3, be3, ms3, Wl, bl):
    plan, nc_ = _get_plan_nc(edge_index, batch)
    NX, SH, TBL, CWP = plan["NX"], plan["SH"], plan["TBL"], plan["CWP"]
    dinv = plan["dinv"]
    r_global = plan["r_global"]

    x = np.asarray(x, np.float32)
    t1_np = np.zeros((TBL + 1, HID), np.float32)
    t1_np[r_global] = (x @ np.asarray(W1, np.float32)) * dinv[:, None]
    t1_np = t1_np.astype(BF)

    prm_np = np.zeros((16, HID), np.float32)
    for i, v in enumerate([b1, b2, b3, be1, be2, be3, g1, g2, g3,
                           ms1, ms2, ms3]):
        prm_np[i] = np.asarray(v, np.float32)
    prm_np[12, :3] = np.asarray(bl, np.float32)

    def blockdiag(w):
        w4 = np.zeros((P, P), np.float32)
        for s in range(4):
            w4[s * HID:(s + 1) * HID, s * HID:(s + 1) * HID] = \
                np.asarray(w, np.float32)
        return w4.astype(BF)

    in_maps = []
    base = dict(
        ones=plan["ones_all"], prm=prm_np,
        pidx=np.tile(np.arange(16, dtype=np.int32), (P, 1)),
        w4_2=blockdiag(W2), w4_3=blockdiag(W3),
        wl=np.asarray(Wl, np.float32),
        idb=np.eye(P, dtype=BF), idf=np.eye(P, dtype=np.float32),
    )
    for c in range(NC):
        # layer-1 messages pre-gathered on host
        mg_c = t1_np[plan["idx_msgs"][c]].reshape(P, CWP * HID)
        in_maps.append(dict(
            base,
            mg=mg_c,
            idxm=plan["idx_msgs"][c],
            idxs=plan["idx_stats"][c],
            idxb=plan["idx_bc"][c],
            dinv=plan["dinv_agg"][c],
            invc=plan["invcnt_col"][c][:, None].astype(np.float32),
        ))
    res = run_bass_kernel_spmd(nc_, in_maps, list(range(NC)))
    if DEBUG:
        kernel._last_results = res.results
    return np.asarray(res.results[0]["out"], np.float32)


# revision 50
# speedup vs baseline: 1.1170x; 1.1170x over previous
"""GCN graph classifier on 8 Trainium2 NeuronCores (Bass/Tile).

Graphs (and their nodes) are sharded across the 8 cores; each layer's
node table x*dinv@W is replicated via AllGather in bf16.  Message
aggregation = indirect-DMA gathers (degree-class packed, one index
column per instruction — the HW's random-gather granularity) +
block-ones PE matmuls that sum each node's messages on the tensor
engine.  Layer-1 messages are pre-gathered on the host (indices are
host-known), so layer 1 streams bulk DMA instead of gathers.
GraphNorm runs as one fused pass: y is bounced to DRAM, per-graph
sums come from one gather set + ones-vector matmuls (y^2 is squared
in SBUF from the same gathered values), all scalar math happens in
graph space, and a single indirect gather broadcasts the fused affine
(A, B) back to the nodes: out = y*A[g] + B[g].
"""
import numpy as np
import ml_dtypes

import concourse.bacc as bacc
import concourse.bass as bass
import concourse.mybir as mybir
import concourse.tile as tile
from concourse.bass_utils import run_bass_kernel_spmd

BF = ml_dtypes.bfloat16

N_NODES = 100000
N_GRAPHS = 1024
HID = 32
EPS = 1e-5
NC = 8
P = 128
GPC = N_GRAPHS // NC          # graphs per core
KP_CLASSES = [(12, 10), (14, 9), (16, 8), (18, 7), (21, 6), (25, 5),
              (32, 4), (42, 3), (64, 2), (128, 1)]
MM_COLS = 16                  # idx-cols (node column-groups) per matmul
GCH = 160                     # idx-cols per gather chunk
LANES = (0, 32, 64)
FG_BANKS = 2                  # psum banks per flush group
LG_MM = FG_BANKS              # matmuls per lane-group
f32 = mybir.dt.float32
bf16 = mybir.dt.bfloat16
i32 = mybir.dt.int32

SIM_INIT_PSUM = False
DEBUG = False


def _preprocess_structure(edge_index, batch):
    ei = np.asarray(edge_index, dtype=np.int64)
    row = np.concatenate([ei[0], np.arange(N_NODES, dtype=np.int64)])
    col = np.concatenate([ei[1], np.arange(N_NODES, dtype=np.int64)])
    batch = np.asarray(batch, dtype=np.int64)

    deg = np.bincount(col, minlength=N_NODES)
    assert deg.max() <= 128, f"max degree {deg.max()} > 128"
    dinv = (1.0 / np.sqrt(np.maximum(deg, 1.0))).astype(np.float32)
    cnt = np.bincount(batch, minlength=N_GRAPHS)
    assert cnt.max() <= 256, f"max graph size {cnt.max()} > 256"
    inv_cnt = (1.0 / np.maximum(cnt, 1.0)).astype(np.float32)

    order = np.argsort(col, kind="stable")
    srcs = row[order]
    indptr = np.zeros(N_NODES + 1, np.int64)
    np.cumsum(np.bincount(col, minlength=N_NODES), out=indptr[1:])

    node_core = (batch // GPC).astype(np.int64)
    core_start = np.searchsorted(batch, np.arange(0, N_GRAPHS + 1, GPC))

    kp_arr = np.array([k for k, _ in KP_CLASSES])
    cls_of = np.searchsorted(kp_arr, deg)

    members = []
    for c in range(NC):
        lo, hi = core_start[c], core_start[c + 1]
        ids = np.arange(lo, hi)
        members.append([ids[cls_of[lo:hi] == k] for k in range(len(KP_CLASSES))])

    cols_k = []
    for k, (kp, npc) in enumerate(KP_CLASSES):
        m = max((len(members[c][k]) + npc - 1) // npc for c in range(NC))
        cols_k.append(-(-m // MM_COLS) * MM_COLS if m else 0)

    mms = []
    for k, (kp, npc) in enumerate(KP_CLASSES):
        for a in range(0, cols_k[k], MM_COLS):
            mms.append((k, a, min(MM_COLS, cols_k[k] - a)))
    n_mm = len(mms)

    descs = []
    R = 0
    B = 0
    for m0 in range(0, n_mm, LG_MM):
        grp = mms[m0:m0 + LG_MM]
        npc_max = max(KP_CLASSES[k][1] for k, _, _ in grp)
        lg = m0 // LG_MM
        lane = LANES[lg % 3]
        fg = lg // 3
        if R + npc_max > P:
            R = 0
            B += FG_BANKS * MM_COLS
        for i, (k, a, ncols) in enumerate(grp):
            descs.append(dict(k=k, a=a, ncols=ncols, npc=KP_CLASSES[k][1],
                              fg=fg, lane=lane, bank=i,
                              aggR=R, band=B + i * MM_COLS))
        R += npc_max
    NX = B + FG_BANKS * MM_COLS
    NX = -(-NX // 4) * 4
    SH = P * NX
    TBL = NC * SH
    CW = sum(cols_k)
    CWP = -(-CW // GCH) * GCH

    agg_p = np.full(N_NODES, -1, np.int32)
    agg_x = np.full(N_NODES, -1, np.int32)
    col_base = np.concatenate([[0], np.cumsum(cols_k)[:-1]]).astype(np.int64)
    for c in range(NC):
        for d in descs:
            k, a, ncols, npc = d["k"], d["a"], d["ncols"], d["npc"]
            mem = members[c][k]
            for j in range(ncols):
                nodes = mem[(a + j) * npc:(a + j + 1) * npc]
                agg_p[nodes] = d["aggR"] + np.arange(len(nodes))
                agg_x[nodes] = d["band"] + j
    r_local = agg_p.astype(np.int64) * NX + agg_x
    r_global = node_core * SH + r_local

    idx_msgs = np.full((NC, P, CWP), TBL, np.int32)
    for c in range(NC):
        for d in descs:
            k, a, ncols, npc = d["k"], d["a"], d["ncols"], d["npc"]
            kp = KP_CLASSES[k][0]
            mem = members[c][k]
            gc0 = col_base[k] + a
            for j in range(ncols):
                nodes = mem[(a + j) * npc:(a + j + 1) * npc]
                for l, v in enumerate(nodes):
                    dv = deg[v]
                    idx_msgs[c, l * kp:l * kp + dv, gc0 + j] = \
                        r_global[srcs[indptr[v]:indptr[v + 1]]]

    msg_col_real = (idx_msgs != TBL).any(axis=(0, 1))        # [CWP]

    # two 128-slot columns per graph (graph sizes can exceed 128)
    idx_stats = np.full((NC, P, 2 * GPC), SH, np.int32)
    for c in range(NC):
        lo, hi = core_start[c], core_start[c + 1]
        b_loc = batch[lo:hi] - c * GPC
        ids = np.arange(lo, hi)
        for g in range(GPC):
            ms = ids[b_loc == g]
            n0 = min(len(ms), P)
            idx_stats[c, :n0, 2 * g] = r_local[ms[:n0]]
            if len(ms) > P:
                idx_stats[c, :len(ms) - P, 2 * g + 1] = r_local[ms[P:]]
    stat_col_real = (idx_stats != SH).any(axis=(0, 1))       # [2*GPC]

    idx_bc = np.full((NC, P, NX), GPC, np.int32)
    dinv_agg = np.zeros((NC, P, NX), np.float32)
    for c in range(NC):
        ids = np.arange(core_start[c], core_start[c + 1])
        idx_bc[c, agg_p[ids], agg_x[ids]] = (batch[ids] - c * GPC).astype(np.int32)
        dinv_agg[c, agg_p[ids], agg_x[ids]] = dinv[ids]

    invcnt_col = inv_cnt.reshape(NC, GPC)

    ones_all = np.zeros((P, sum(n for _, n in KP_CLASSES)), BF)
    ones_off = []
    off = 0
    for kp, npc in KP_CLASSES:
        ones_off.append(off)
        for l in range(npc):
            ones_all[l * kp:(l + 1) * kp, off + l] = 1
        off += npc

    return dict(
        deg=deg, dinv=dinv, inv_cnt=inv_cnt,
        descs=descs, cols_k=cols_k, col_base=col_base, NX=NX, SH=SH,
        TBL=TBL, CW=CW, CWP=CWP,
        r_global=r_global, r_local=r_local, agg_p=agg_p, agg_x=agg_x,
        core_start=core_start, msg_col_real=msg_col_real,
        stat_col_real=stat_col_real,
        idx_msgs=idx_msgs, idx_stats=idx_stats, idx_bc=idx_bc,
        dinv_agg=dinv_agg, invcnt_col=invcnt_col,
        ones_all=ones_all, ones_off=ones_off,
    )


def _build(plan):
    NX, SH, TBL, CWP = plan["NX"], plan["SH"], plan["TBL"], plan["CWP"]
    NXF = NX * HID
    NB = NX // 4
    NONES = plan["ones_all"].shape[1]
    NST = 2 * GPC * HID // 512    # stats matmuls per source (=16)
    msg_col_real = plan["msg_col_real"]
    stat_col_real = plan["stat_col_real"]

    nc_ = bacc.Bacc(None, target_bir_lowering=False)

    mg = nc_.declare_dram_parameter("mg", [P, CWP * HID], bf16, isOutput=False)
    idxm = nc_.declare_dram_parameter("idxm", [P, CWP], i32, isOutput=False)
    idxs = nc_.declare_dram_parameter("idxs", [P, 2 * GPC], i32, isOutput=False)
    idxb = nc_.declare_dram_parameter("idxb", [P, NX], i32, isOutput=False)
    dinv_in = nc_.declare_dram_parameter("dinv", [P, NX], f32, isOutput=False)
    invc_in = nc_.declare_dram_parameter("invc", [P, 1], f32, isOutput=False)
    ones_in = nc_.declare_dram_parameter("ones", [P, NONES], bf16, isOutput=False)
    prm_in = nc_.declare_dram_parameter("prm", [16, HID], f32, isOutput=False)
    pidx_in = nc_.declare_dram_parameter("pidx", [P, 16], i32, isOutput=False)
    w4_2_in = nc_.declare_dram_parameter("w4_2", [P, P], bf16, isOutput=False)
    w4_3_in = nc_.declare_dram_parameter("w4_3", [P, P], bf16, isOutput=False)
    wl_in = nc_.declare_dram_parameter("wl", [HID, 3], f32, isOutput=False)
    idb_in = nc_.declare_dram_parameter("idb", [P, P], bf16, isOutput=False)
    idf_in = nc_.declare_dram_parameter("idf", [P, P], f32, isOutput=False)
    out_t = nc_.declare_dram_parameter("out", [N_GRAPHS, 3], f32, isOutput=True)

    tabA = nc_.dram_tensor("tabA", [TBL + 1, HID], bf16)
    tabB = nc_.dram_tensor("tabB", [TBL + 1, HID], bf16)
    stg = nc_.dram_tensor("stg", [SH, HID], bf16)
    xb1 = nc_.dram_tensor("xb1", [SH + 1, HID], bf16)
    cbd = nc_.dram_tensor("cbd", [GPC + 1, 2 * HID], f32)
    srd = nc_.dram_tensor("srd", [4 * GPC, HID], f32)
    lg_in = nc_.dram_tensor("lg_in", [GPC, 3], f32)
    lg_out = nc_.dram_tensor("lg_out", [N_GRAPHS, 3], f32)

    RG = [list(range(NC))]
    AX = mybir.AluOpType
    ACT = mybir.ActivationFunctionType

    with tile.TileContext(nc_) as tc:
        with (
            tc.tile_pool(name="persist", bufs=1) as pp,
            tc.tile_pool(name="work", bufs=1) as wp,
            tc.tile_pool(name="gather", bufs=2) as gp,
            tc.tile_pool(name="stat", bufs=1) as sp,
        ):
            idxm_t = pp.tile([P, CWP], i32)
            nc_.sync.dma_start(out=idxm_t[:], in_=idxm[:, :])
            idxs_t = pp.tile([P, 2 * GPC], i32)
            nc_.sync.dma_start(out=idxs_t[:], in_=idxs[:, :])
            idxb_t = pp.tile([P, NX], i32)
            nc_.sync.dma_start(out=idxb_t[:], in_=idxb[:, :])
            dinv_t = pp.tile([P, NX], f32)
            nc_.sync.dma_start(out=dinv_t[:], in_=dinv_in[:, :])
            dinv_b = pp.tile([P, NX], bf16)
            nc_.vector.tensor_copy(out=dinv_b[:], in_=dinv_t[:])
            invc_t = pp.tile([P, 1], f32)
            nc_.sync.dma_start(out=invc_t[:], in_=invc_in[:, :])
            ones_t = pp.tile([P, NONES], bf16)
            nc_.sync.dma_start(out=ones_t[:], in_=ones_in[:, :])
            w42_t = pp.tile([P, P], bf16)
            nc_.sync.dma_start(out=w42_t[:], in_=w4_2_in[:, :])
            w43_t = pp.tile([P, P], bf16)
            nc_.sync.dma_start(out=w43_t[:], in_=w4_3_in[:, :])
            wl_t = pp.tile([HID, 3], f32)
            nc_.sync.dma_start(out=wl_t[:], in_=wl_in[:, :])
            idb_t = pp.tile([P, P], bf16)
            nc_.sync.dma_start(out=idb_t[:], in_=idb_in[:, :])
            idf_t = pp.tile([P, P], f32)
            nc_.sync.dma_start(out=idf_t[:], in_=idf_in[:, :])
            pidx_t = pp.tile([P, 16], i32)
            nc_.sync.dma_start(out=pidx_t[:], in_=pidx_in[:, :])
            prm_t = pp.tile([P, 16 * HID], f32)
            for jj in range(16):
                nc_.gpsimd.indirect_dma_start(
                    out=prm_t[:, jj * HID:(jj + 1) * HID],
                    out_offset=None, in_=prm_in[:],
                    in_offset=bass.IndirectOffsetOnAxis(
                        ap=pidx_t[:, jj:jj + 1], axis=0))
            ocol_t = pp.tile([P, 1], bf16)
            nc_.vector.memset(ocol_t[:], 1.0)

            zrow = pp.tile([1, 2 * HID], f32)
            nc_.vector.memset(zrow[:], 0.0)
            zrow_b = pp.tile([1, HID], bf16)
            nc_.vector.memset(zrow_b[:], 0.0)
            nc_.sync.dma_start(out=tabA[TBL:TBL + 1, :], in_=zrow_b[:])
            nc_.sync.dma_start(out=tabB[TBL:TBL + 1, :], in_=zrow_b[:])
            nc_.sync.dma_start(out=xb1[SH:SH + 1, :], in_=zrow_b[:])
            nc_.sync.dma_start(out=cbd[GPC:GPC + 1, :], in_=zrow[:])

            def prm_row(r):
                return prm_t[:, r * HID:(r + 1) * HID]

            def as3(ap):
                return ap.rearrange("p (x f) -> p x f", f=HID)

            def stats_sums(bounce, tag, want_sq=True):
                """bounce [SH+1] bf16 -> (s1, s2) per-graph sums of y and
                y^2, [GPC, HID] f32 each.  One gather set (real columns
                only, padded columns zeroed by memset); y^2 squared in
                SBUF from the gathered values."""
                st = sp.tile([P, 2 * GPC * HID], bf16, tag="stbuf")
                nc_.vector.memset(st[:], 0.0)
                for jj in range(2 * GPC):
                    if stat_col_real[jj]:
                        nc_.gpsimd.indirect_dma_start(
                            out=st[:, jj * HID:(jj + 1) * HID],
                            out_offset=None, in_=bounce[:],
                            in_offset=bass.IndirectOffsetOnAxis(
                                ap=idxs_t[:, jj:jj + 1], axis=0))
                if want_sq:
                    stq = sp.tile([P, 2 * GPC * HID], bf16, tag="stq")
                    nc_.vector.tensor_tensor(out=stq[:], in0=st[:], in1=st[:],
                                             op=AX.mult)

                def half(src, blk):
                    srow = wp.tile([1, NST * 512], f32, tag="srow")
                    with tc.tile_pool(name=f"psS{tag}{blk}", bufs=2,
                                      space="PSUM") as psS:
                        for m in range(NST):
                            pss = psS.tile([1, 512], f32, space="PSUM",
                                           tag="sps")
                            nc_.tensor.matmul(
                                out=pss[:], lhsT=ocol_t[:],
                                rhs=src[:, m * 512:(m + 1) * 512],
                                start=True, stop=True)
                            dst = srow[:, m * 512:(m + 1) * 512]
                            if m % 2 == 0:
                                nc_.scalar.activation(out=dst, in_=pss[:],
                                                      func=ACT.Copy)
                            else:
                                nc_.vector.tensor_copy(out=dst, in_=pss[:])
                    nc_.sync.dma_start(
                        out=srd[blk * 2 * GPC:(blk + 1) * 2 * GPC, :],
                        in_=srow[:])
                    sw = wp.tile([GPC, 2 * HID], f32, tag=f"sw{blk}")
                    nc_.sync.dma_start(
                        out=sw[:],
                        in_=srd[blk * 2 * GPC:(blk + 1) * 2 * GPC, :]
                        .rearrange("(g two) f -> g (two f)", two=2))
                    s = wp.tile([GPC, HID], f32, tag=tag + f"s{blk}")
                    nc_.vector.tensor_tensor(out=s[:], in0=sw[:, 0:HID],
                                             in1=sw[:, HID:2 * HID],
                                             op=AX.add)
                    return s

                s1 = half(st, 0)
                s2 = half(stq, 1) if want_sq else None
                return s1, s2

            descs = plan["descs"]
            col_base = plan["col_base"]
            ones_off = plan["ones_off"]
            tables = [None, tabA, tabB]
            x_prev = {}

            for L in range(3):
                agg = wp.tile([P, NXF], f32, tag="agg")
                nc_.vector.memset(agg[:], 0.0)
                # ---------- aggregation ----------
                gbufs = {}
                with tc.tile_pool(name=f"psA{L}", bufs=2, space="PSUM") as psA:
                    ps = None
                    fg_list = []
                    last_fg = -1

                    def flush(ps, fg_list):
                        sc = wp.tile([P, FG_BANKS * 512], f32, tag="scr")
                        nc_.scalar.activation(out=sc[:], in_=ps[:],
                                              func=ACT.Copy)
                        for dd in fg_list:
                            nc_.sync.dma_start(
                                out=agg[dd["aggR"]:dd["aggR"] + dd["npc"],
                                        dd["band"] * HID:
                                        (dd["band"] + dd["ncols"]) * HID],
                                in_=sc[dd["lane"]:dd["lane"] + dd["npc"],
                                       dd["bank"] * 512:
                                       dd["bank"] * 512 + dd["ncols"] * HID])

                    for m, d in enumerate(descs):
                        gcol = int(col_base[d["k"]] + d["a"])
                        ch = gcol // GCH
                        if ch not in gbufs:
                            gb = gp.tile([P, GCH * HID], bf16, tag="gbuf")
                            if L == 0:
                                nc_.sync.dma_start(
                                    out=gb[:],
                                    in_=mg[:, ch * GCH * HID:
                                           (ch + 1) * GCH * HID])
                            else:
                                for j in range(GCH):
                                    jc = ch * GCH + j
                                    if jc < CWP and msg_col_real[jc]:
                                        nc_.gpsimd.indirect_dma_start(
                                            out=gb[:, j * HID:(j + 1) * HID],
                                            out_offset=None,
                                            in_=tables[L][:],
                                            in_offset=bass.IndirectOffsetOnAxis(
                                                ap=idxm_t[:, jc:jc + 1],
                                                axis=0))
                            gbufs[ch] = gb
                        if d["fg"] != last_fg:
                            if ps is not None:
                                flush(ps, fg_list)
                            ps = psA.tile([P, FG_BANKS * 512], f32,
                                          space="PSUM", tag="aggps")
                            if SIM_INIT_PSUM:
                                nc_.vector.memset(ps[:], 0.0)
                            fg_list = []
                            last_fg = d["fg"]
                        loc = gcol - ch * GCH
                        off = ones_off[d["k"]]
                        nc_.tensor.matmul(
                            out=ps[d["lane"]:d["lane"] + d["npc"],
                                   d["bank"] * 512:
                                   d["bank"] * 512 + d["ncols"] * HID],
                            lhsT=ones_t[:, off:off + d["npc"]],
                            rhs=gbufs[ch][:, loc * HID:(loc + d["ncols"]) * HID],
                            start=True, stop=True)
                        fg_list.append(d)
                    flush(ps, fg_list)

                # ---------- y = agg * dinv ----------
                nc_.vector.tensor_tensor(
                    out=as3(agg[:]), in0=as3(agg[:]),
                    in1=dinv_t[:][:, :, None].to_broadcast([P, NX, HID]),
                    op=AX.mult)

                # ---------- graph norm (fused single pass) ----------
                nc_.gpsimd.dma_start(   # cast f32->bf16 in flight
                    out=xb1[0:SH, :].rearrange("(p x) f -> p (x f)", p=P),
                    in_=agg[:])
                s1, s2 = stats_sums(xb1, f"st{L}")

                # graph-space math: x = y + b;  m = E[x]; e2 = E[x^2]
                my = wp.tile([GPC, HID], f32, tag="my")
                nc_.vector.tensor_scalar_mul(out=my[:], in0=s1[:],
                                             scalar1=invc_t[:, 0:1])
                mm = wp.tile([GPC, HID], f32, tag="mm")
                nc_.vector.tensor_tensor(out=mm[:], in0=my[:],
                                         in1=prm_row(0 + L), op=AX.add)
                e2 = wp.tile([GPC, HID], f32, tag="e2")
                nc_.vector.tensor_scalar(out=e2[:], in0=s2[:],
                                         scalar1=invc_t[:, 0:1], scalar2=EPS,
                                         op0=AX.mult, op1=AX.add)
                u = wp.tile([GPC, HID], f32, tag="u")
                nc_.vector.tensor_tensor(out=u[:], in0=my[:], in1=mm[:],
                                         op=AX.add)
                nc_.vector.tensor_tensor(out=u[:], in0=u[:],
                                         in1=prm_row(0 + L), op=AX.mult)
                nc_.vector.tensor_tensor(out=e2[:], in0=e2[:], in1=u[:],
                                         op=AX.add)
                msm = wp.tile([GPC, HID], f32, tag="msm")
                nc_.vector.tensor_tensor(out=msm[:], in0=mm[:],
                                         in1=prm_row(9 + L), op=AX.mult)
                r = wp.tile([GPC, HID], f32, tag="r")
                nc_.vector.tensor_tensor(out=r[:], in0=mm[:], in1=msm[:],
                                         op=AX.subtract)
                nc_.vector.tensor_tensor(out=r[:], in0=r[:], in1=mm[:],
                                         op=AX.add)
                nc_.vector.tensor_tensor(out=r[:], in0=r[:], in1=msm[:],
                                         op=AX.mult)
                var = wp.tile([GPC, HID], f32, tag="var")
                nc_.vector.tensor_tensor(out=var[:], in0=e2[:], in1=r[:],
                                         op=AX.subtract)
                rstd = wp.tile([GPC, HID], f32, tag="rstd")
                nc_.vector.reciprocal(out=rstd[:], in_=var[:])
                nc_.scalar.activation(out=rstd[:], in_=rstd[:], func=ACT.Sqrt)
                ab = wp.tile([GPC, 2 * HID], f32, tag="ab")
                nc_.vector.tensor_tensor(out=ab[:, 0:HID], in0=rstd[:],
                                         in1=prm_row(6 + L), op=AX.mult)
                d2 = wp.tile([GPC, HID], f32, tag="d2")
                nc_.vector.tensor_tensor(out=d2[:], in0=prm_row(0 + L),
                                         in1=msm[:], op=AX.subtract)
                nc_.vector.tensor_tensor(out=d2[:], in0=d2[:],
                                         in1=ab[:, 0:HID], op=AX.mult)
                nc_.vector.tensor_tensor(out=ab[:, HID:2 * HID], in0=d2[:],
                                         in1=prm_row(3 + L), op=AX.add)
                # broadcast AB to node slots (256B row per graph)
                nc_.sync.dma_start(out=cbd[0:GPC, :], in_=ab[:])
                outb = wp.tile([P, NX * 2 * HID], f32, tag="bcbuf")
                for x in range(NX):
                    nc_.gpsimd.indirect_dma_start(
                        out=outb[:, x * 2 * HID:(x + 1) * 2 * HID],
                        out_offset=None, in_=cbd[:],
                        in_offset=bass.IndirectOffsetOnAxis(
                            ap=idxb_t[:, x:x + 1], axis=0))
                obv = outb[:].rearrange("p (x tf) -> p x tf", tf=2 * HID)
                # fused: xl = relu(y*A + B (+ x_prev))
                nc_.vector.tensor_tensor(out=as3(agg[:]), in0=as3(agg[:]),
                                         in1=obv[:, :, 0:HID], op=AX.mult)
                nc_.vector.tensor_tensor(out=as3(agg[:]), in0=as3(agg[:]),
                                         in1=obv[:, :, HID:2 * HID],
                                         op=AX.add)
                xl = wp.tile([P, NXF], bf16, tag=f"x{L % 2}")
                if L > 0:
                    nc_.vector.tensor_copy(out=xl[:], in_=agg[:])
                    nc_.vector.tensor_tensor(out=xl[:], in0=xl[:],
                                             in1=x_prev[L - 1][:], op=AX.add)
                    nc_.scalar.activation(out=xl[:], in_=xl[:], func=ACT.Relu)
                else:
                    nc_.scalar.activation(out=xl[:], in_=agg[:], func=ACT.Relu)
                x_prev[L] = xl

                if L < 2:
                    # ---------- next table ----------
                    w4 = w42_t if L == 0 else w43_t
                    tab = tabA if L == 0 else tabB
                    yb = wp.tile([P, NXF], bf16, tag="yb")
                    nc_.vector.tensor_tensor(
                        out=as3(yb[:]), in0=as3(xl[:]),
                        in1=dinv_b[:][:, :, None].to_broadcast([P, NX, HID]),
                        op=AX.mult)
                    xt4 = wp.tile([P, NXF], bf16, tag="xt4")
                    with tc.tile_pool(name=f"psT{L}", bufs=2, space="PSUM") as psT:
                        for b in range(NB):
                            pst = psT.tile([P, P], bf16, space="PSUM", tag="tp")
                            nc_.tensor.transpose(out=pst[:],
                                                 in_=yb[:, b * P:(b + 1) * P],
                                                 identity=idb_t[:])
                            nc_.vector.tensor_copy(
                                out=xt4[:, b * P:(b + 1) * P], in_=pst[:])
                    zt4 = wp.tile([P, NXF], bf16, tag="zt4")
                    with tc.tile_pool(name=f"psW{L}", bufs=2, space="PSUM") as psW:
                        for q in range(NXF // 512):
                            psw = psW.tile([P, 512], f32, space="PSUM", tag="wp")
                            nc_.tensor.matmul(out=psw[:], lhsT=w4[:],
                                              rhs=xt4[:, q * 512:(q + 1) * 512],
                                              start=True, stop=True)
                            nc_.vector.tensor_copy(
                                out=zt4[:, q * 512:(q + 1) * 512], in_=psw[:])
                    zb = wp.tile([P, NXF], bf16, tag="zb")
                    with tc.tile_pool(name=f"psU{L}", bufs=2, space="PSUM") as psU:
                        for b in range(NB):
                            pst = psU.tile([P, P], bf16, space="PSUM", tag="tp2")
                            nc_.tensor.transpose(out=pst[:],
                                                 in_=zt4[:, b * P:(b + 1) * P],
                                                 identity=idb_t[:])
                            nc_.scalar.activation(
                                out=zb[:, b * P:(b + 1) * P], in_=pst[:],
                                func=ACT.Copy)
                    nc_.sync.dma_start(
                        out=stg[:, :].rearrange("(p x) f -> p (x f)", p=P),
                        in_=zb[:])
                    nc_.gpsimd.collective_compute(
                        "AllGather", AX.bypass, replica_groups=RG,
                        ins=[stg[:, :]], outs=[tab[0:TBL, :]])
                else:
                    # ---------- pool + logits ----------
                    nc_.sync.dma_start(
                        out=xb1[0:SH, :].rearrange("(p x) f -> p (x f)", p=P),
                        in_=xl[:])
                    s3, _s3q = stats_sums(xb1, "stF", want_sq=False)
                    pooled = wp.tile([GPC, HID], f32, tag="pooled")
                    nc_.vector.tensor_scalar_mul(out=pooled[:], in0=s3[:],
                                                 scalar1=invc_t[:, 0:1])
                    with tc.tile_pool(name="psF", bufs=1, space="PSUM") as psF:
                        pstp = psF.tile([HID, P], f32, space="PSUM", tag="pt")
                        nc_.tensor.transpose(out=pstp[:], in_=pooled[:],
                                             identity=idf_t[:])
                        pooledT = wp.tile([HID, P], f32, tag="pooledT")
                        nc_.vector.tensor_copy(out=pooledT[:], in_=pstp[:])
                        psl = psF.tile([P, 3], f32, space="PSUM", tag="lg")
                        nc_.tensor.matmul(out=psl[:], lhsT=pooledT[:],
                                          rhs=wl_t[:], start=True, stop=True)
                        logits = wp.tile([GPC, 3], f32, tag="logits")
                        nc_.vector.tensor_tensor(
                            out=logits[:], in0=psl[:],
                            in1=prm_t[0:GPC, 12 * HID:12 * HID + 3], op=AX.add)
                    nc_.sync.dma_start(out=lg_in[:, :], in_=logits[:])
                    nc_.gpsimd.collective_compute(
                        "AllGather", AX.bypass, replica_groups=RG,
                        ins=[lg_in[:, :]], outs=[lg_out[:, :]])
                    ofin = wp.tile([P, N_GRAPHS * 3 // P], f32, tag="ofin")
                    nc_.sync.dma_start(
                        out=ofin[:],
                        in_=lg_out[:, :].rearrange("(p a) f -> p (a f)", p=P))
                    nc_.sync.dma_start(
                        out=out_t[:, :].rearrange("(p a) f -> p (a f)", p=P),
                        in_=ofin[:])

    nc_.finalize()
    return nc_


_CACHE = {}


def _get_plan_nc(edge_index, batch):
    key = (hash(np.asarray(edge_index)[:, ::997].tobytes()),
           hash(np.asarray(batch)[::97].tobytes()))
    if key not in _CACHE:
        plan = _preprocess_structure(edge_index, batch)
        nc_ = _build(plan)
        _CACHE[key] = (plan, nc_)
    return _CACHE[key]


def kernel(x, edge_index, batch, W1, b1, W2, b2, W3, b3,
           g1, be1, ms1, g2, be2, ms2, g3, be3, ms3, Wl, bl):
    plan, nc_ = _get_plan_nc(edge_index, batch)
    NX, SH, TBL, CWP = plan["NX"], plan["SH"], plan["TBL"], plan["CWP"]
    dinv = plan["dinv"]
    r_global = plan["r_global"]

    x = np.asarray(x, np.float32)
    t1_np = np.zeros((TBL + 1, HID), np.float32)
    t1_np[r_global] = (x @ np.asarray(W1, np.float32)) * dinv[:, None]
    t1_np = t1_np.astype(BF)

    prm_np = np.zeros((16, HID), np.float32)
    for i, v in enumerate([b1, b2, b3, be1, be2, be3, g1, g2, g3,
                           ms1, ms2, ms3]):
        prm_np[i] = np.asarray(v, np.float32)
    prm_np[12, :3] = np.asarray(bl, np.float32)

    def blockdiag(w):
        w4 = np.zeros((P, P), np.float32)
        for s in range(4):
            w4[s * HID:(s + 1) * HID, s * HID:(s + 1) * HID] = \
                np.asarray(w, np.float32)
        return w4.astype(BF)

    in_maps = []
    base = dict(
        ones=plan["ones_all"], prm=prm_np,
        pidx=np.tile(np.arange(16, dtype=np.int32), (P, 1)),
        w4_2=blockdiag(W2), w4_3=blockdiag(W3),
        wl=np.asarray(Wl, np.float32),
        idb=np.eye(P, dtype=BF), idf=np.eye(P, dtype=np.float32),
    )
    for c in range(NC):
        # layer-1 messages pre-gathered on host
        mg_c = t1_np[plan["idx_msgs"][c]].reshape(P, CWP * HID)
        in_maps.append(dict(
            base,
            mg=mg_c,
            idxm=plan["idx_msgs"][c],
            idxs=plan["idx_stats"][c],
            idxb=plan["idx_bc"][c],
            dinv=plan["dinv_agg"][c],
            invc=plan["invcnt_col"][c][:, None].astype(np.float32),
        ))
    res = run_bass_kernel_spmd(nc_, in_maps, list(range(NC)))
    if DEBUG:
        kernel._last_results = res.results
    return np.asarray(res.results[0]["out"], np.float32)


# revision 57
# speedup vs baseline: 1.1686x; 1.0462x over previous
"""GCN graph classifier on 8 Trainium2 NeuronCores (Bass/Tile).

Graphs (and their nodes) are sharded across the 8 cores; each layer's
node table x*dinv@W is replicated via AllGather in bf16.  Message
aggregation = indirect-DMA gathers (degree-class packed, one index
column per instruction — the HW's random-gather granularity) +
block-ones PE matmuls that sum each node's messages on the tensor
engine.  Layer-1 messages are pre-gathered on the host (indices are
host-known), so layer 1 streams bulk DMA instead of gathers.
GraphNorm runs as one fused pass: y is bounced to DRAM, per-graph
sums come from one gather set + ones-vector matmuls (y^2 is squared
in SBUF from the same gathered values), all scalar math happens in
graph space, and a single indirect gather broadcasts the fused affine
(A, B) back to the nodes: out = y*A[g] + B[g].
"""
import numpy as np
import ml_dtypes

import concourse.bacc as bacc
import concourse.bass as bass
import concourse.mybir as mybir
import concourse.tile as tile
from concourse.bass_utils import run_bass_kernel_spmd

BF = ml_dtypes.bfloat16

N_NODES = 100000
N_GRAPHS = 1024
HID = 32
EPS = 1e-5
NC = 8
P = 128
GPC = N_GRAPHS // NC          # graphs per core
KP_CLASSES = [(12, 10), (14, 9), (16, 8), (18, 7), (21, 6), (25, 5),
              (32, 4), (42, 3), (64, 2), (128, 1)]
MM_COLS = 16                  # idx-cols (node column-groups) per matmul
GCH = 160                     # idx-cols per gather chunk
LANES = (0, 32, 64)
FG_BANKS = 2                  # psum banks per flush group
LG_MM = FG_BANKS              # matmuls per lane-group
f32 = mybir.dt.float32
bf16 = mybir.dt.bfloat16
i32 = mybir.dt.int32

SIM_INIT_PSUM = False
DEBUG = False


def _preprocess_structure(edge_index, batch):
    ei = np.asarray(edge_index, dtype=np.int64)
    row = np.concatenate([ei[0], np.arange(N_NODES, dtype=np.int64)])
    col = np.concatenate([ei[1], np.arange(N_NODES, dtype=np.int64)])
    batch = np.asarray(batch, dtype=np.int64)

    deg = np.bincount(col, minlength=N_NODES)
    assert deg.max() <= 128, f"max degree {deg.max()} > 128"
    dinv = (1.0 / np.sqrt(np.maximum(deg, 1.0))).astype(np.float32)
    cnt = np.bincount(batch, minlength=N_GRAPHS)
    assert cnt.max() <= 256, f"max graph size {cnt.max()} > 256"
    inv_cnt = (1.0 / np.maximum(cnt, 1.0)).astype(np.float32)

    order = np.argsort(col, kind="stable")
    srcs = row[order]
    indptr = np.zeros(N_NODES + 1, np.int64)
    np.cumsum(np.bincount(col, minlength=N_NODES), out=indptr[1:])

    node_core = (batch // GPC).astype(np.int64)
    core_start = np.searchsorted(batch, np.arange(0, N_GRAPHS + 1, GPC))

    # the appended self-loop (last entry per node — argsort is stable and
    # loops were appended after edge_index) is served from the locally
    # staged table tile instead of a gather, so slots per node = deg-1
    mcnt = deg - 1
    kp_arr = np.array([k for k, _ in KP_CLASSES])
    cls_of = np.searchsorted(kp_arr, mcnt)

    members = []
    for c in range(NC):
        lo, hi = core_start[c], core_start[c + 1]
        ids = np.arange(lo, hi)
        members.append([ids[cls_of[lo:hi] == k] for k in range(len(KP_CLASSES))])

    cols_k = []
    for k, (kp, npc) in enumerate(KP_CLASSES):
        m = max((len(members[c][k]) + npc - 1) // npc for c in range(NC))
        cols_k.append(-(-m // MM_COLS) * MM_COLS if m else 0)

    mms = []
    for k, (kp, npc) in enumerate(KP_CLASSES):
        for a in range(0, cols_k[k], MM_COLS):
            mms.append((k, a, min(MM_COLS, cols_k[k] - a)))
    n_mm = len(mms)

    descs = []
    R = 0
    B = 0
    for m0 in range(0, n_mm, LG_MM):
        grp = mms[m0:m0 + LG_MM]
        npc_max = max(KP_CLASSES[k][1] for k, _, _ in grp)
        lg = m0 // LG_MM
        lane = LANES[lg % 3]
        fg = lg // 3
        if R + npc_max > P:
            R = 0
            B += FG_BANKS * MM_COLS
        for i, (k, a, ncols) in enumerate(grp):
            descs.append(dict(k=k, a=a, ncols=ncols, npc=KP_CLASSES[k][1],
                              fg=fg, lane=lane, bank=i,
                              aggR=R, band=B + i * MM_COLS))
        R += npc_max
    NX = B + FG_BANKS * MM_COLS
    NX = -(-NX // 4) * 4
    SH = P * NX
    TBL = NC * SH
    CW = sum(cols_k)
    CWP = -(-CW // GCH) * GCH

    agg_p = np.full(N_NODES, -1, np.int32)
    agg_x = np.full(N_NODES, -1, np.int32)
    col_base = np.concatenate([[0], np.cumsum(cols_k)[:-1]]).astype(np.int64)
    for c in range(NC):
        for d in descs:
            k, a, ncols, npc = d["k"], d["a"], d["ncols"], d["npc"]
            mem = members[c][k]
            for j in range(ncols):
                nodes = mem[(a + j) * npc:(a + j + 1) * npc]
                agg_p[nodes] = d["aggR"] + np.arange(len(nodes))
                agg_x[nodes] = d["band"] + j
    r_local = agg_p.astype(np.int64) * NX + agg_x
    r_global = node_core * SH + r_local

    idx_msgs = np.full((NC, P, CWP), TBL, np.int32)
    for c in range(NC):
        for d in descs:
            k, a, ncols, npc = d["k"], d["a"], d["ncols"], d["npc"]
            kp = KP_CLASSES[k][0]
            mem = members[c][k]
            gc0 = col_base[k] + a
            for j in range(ncols):
                nodes = mem[(a + j) * npc:(a + j + 1) * npc]
                for l, v in enumerate(nodes):
                    dv = mcnt[v]
                    idx_msgs[c, l * kp:l * kp + dv, gc0 + j] = \
                        r_global[srcs[indptr[v]:indptr[v + 1] - 1]]

    msg_col_real = (idx_msgs != TBL).any(axis=(0, 1))        # [CWP]

    # two 128-slot columns per graph (graph sizes can exceed 128)
    idx_stats = np.full((NC, P, 2 * GPC), SH, np.int32)
    for c in range(NC):
        lo, hi = core_start[c], core_start[c + 1]
        b_loc = batch[lo:hi] - c * GPC
        ids = np.arange(lo, hi)
        for g in range(GPC):
            ms = ids[b_loc == g]
            n0 = min(len(ms), P)
            idx_stats[c, :n0, 2 * g] = r_local[ms[:n0]]
            if len(ms) > P:
                idx_stats[c, :len(ms) - P, 2 * g + 1] = r_local[ms[P:]]
    stat_col_real = (idx_stats != SH).any(axis=(0, 1))       # [2*GPC]

    idx_bc = np.full((NC, P, NX), GPC, np.int32)
    dinv_agg = np.zeros((NC, P, NX), np.float32)
    for c in range(NC):
        ids = np.arange(core_start[c], core_start[c + 1])
        idx_bc[c, agg_p[ids], agg_x[ids]] = (batch[ids] - c * GPC).astype(np.int32)
        dinv_agg[c, agg_p[ids], agg_x[ids]] = dinv[ids]

    invcnt_col = inv_cnt.reshape(NC, GPC)

    ones_all = np.zeros((P, sum(n for _, n in KP_CLASSES)), BF)
    ones_off = []
    off = 0
    for kp, npc in KP_CLASSES:
        ones_off.append(off)
        for l in range(npc):
            ones_all[l * kp:(l + 1) * kp, off + l] = 1
        off += npc

    return dict(
        deg=deg, dinv=dinv, inv_cnt=inv_cnt,
        descs=descs, cols_k=cols_k, col_base=col_base, NX=NX, SH=SH,
        TBL=TBL, CW=CW, CWP=CWP,
        r_global=r_global, r_local=r_local, agg_p=agg_p, agg_x=agg_x,
        core_start=core_start, msg_col_real=msg_col_real,
        stat_col_real=stat_col_real,
        idx_msgs=idx_msgs, idx_stats=idx_stats, idx_bc=idx_bc,
        dinv_agg=dinv_agg, invcnt_col=invcnt_col,
        ones_all=ones_all, ones_off=ones_off,
    )


def _build(plan):
    NX, SH, TBL, CWP = plan["NX"], plan["SH"], plan["TBL"], plan["CWP"]
    NXF = NX * HID
    NB = NX // 4
    NONES = plan["ones_all"].shape[1]
    NST = 2 * GPC * HID // 512    # stats matmuls per source (=16)
    msg_col_real = plan["msg_col_real"]
    stat_col_real = plan["stat_col_real"]

    nc_ = bacc.Bacc(None, target_bir_lowering=False)

    mg = nc_.declare_dram_parameter("mg", [P, CWP * HID], bf16, isOutput=False)
    slf = nc_.declare_dram_parameter("slf", [P, NXF], bf16, isOutput=False)
    idxm = nc_.declare_dram_parameter("idxm", [P, CWP], i32, isOutput=False)
    idxs = nc_.declare_dram_parameter("idxs", [P, 2 * GPC], i32, isOutput=False)
    idxb = nc_.declare_dram_parameter("idxb", [P, NX], i32, isOutput=False)
    dinv_in = nc_.declare_dram_parameter("dinv", [P, NX], f32, isOutput=False)
    invc_in = nc_.declare_dram_parameter("invc", [P, 1], f32, isOutput=False)
    ones_in = nc_.declare_dram_parameter("ones", [P, NONES], bf16, isOutput=False)
    prm_in = nc_.declare_dram_parameter("prm", [16, HID], f32, isOutput=False)
    pidx_in = nc_.declare_dram_parameter("pidx", [P, 16], i32, isOutput=False)
    w4_2_in = nc_.declare_dram_parameter("w4_2", [P, P], bf16, isOutput=False)
    w4_3_in = nc_.declare_dram_parameter("w4_3", [P, P], bf16, isOutput=False)
    wl_in = nc_.declare_dram_parameter("wl", [HID, 3], f32, isOutput=False)
    idb_in = nc_.declare_dram_parameter("idb", [P, P], bf16, isOutput=False)
    idf_in = nc_.declare_dram_parameter("idf", [P, P], f32, isOutput=False)
    out_t = nc_.declare_dram_parameter("out", [N_GRAPHS, 3], f32, isOutput=True)

    tabA = nc_.dram_tensor("tabA", [TBL + 1, HID], bf16)
    tabB = nc_.dram_tensor("tabB", [TBL + 1, HID], bf16)
    stg = nc_.dram_tensor("stg", [SH, HID], bf16)
    xb1 = nc_.dram_tensor("xb1", [SH + 1, HID], bf16)
    cbd = nc_.dram_tensor("cbd", [GPC + 1, 2 * HID], f32)
    srd = nc_.dram_tensor("srd", [4 * GPC, HID], f32)
    lg_in = nc_.dram_tensor("lg_in", [GPC, 3], f32)
    lg_out = nc_.dram_tensor("lg_out", [N_GRAPHS, 3], f32)

    RG = [list(range(NC))]
    AX = mybir.AluOpType
    ACT = mybir.ActivationFunctionType

    with tile.TileContext(nc_) as tc:
        with (
            tc.tile_pool(name="persist", bufs=1) as pp,
            tc.tile_pool(name="work", bufs=1) as wp,
            tc.tile_pool(name="gather", bufs=2) as gp,
            tc.tile_pool(name="stat", bufs=1) as sp,
        ):
            idxm_t = pp.tile([P, CWP], i32)
            nc_.sync.dma_start(out=idxm_t[:], in_=idxm[:, :])
            idxs_t = pp.tile([P, 2 * GPC], i32)
            nc_.sync.dma_start(out=idxs_t[:], in_=idxs[:, :])
            idxb_t = pp.tile([P, NX], i32)
            nc_.sync.dma_start(out=idxb_t[:], in_=idxb[:, :])
            dinv_t = pp.tile([P, NX], f32)
            nc_.sync.dma_start(out=dinv_t[:], in_=dinv_in[:, :])
            dinv_b = pp.tile([P, NX], bf16)
            nc_.vector.tensor_copy(out=dinv_b[:], in_=dinv_t[:])
            invc_t = pp.tile([P, 1], f32)
            nc_.sync.dma_start(out=invc_t[:], in_=invc_in[:, :])
            ones_t = pp.tile([P, NONES], bf16)
            nc_.sync.dma_start(out=ones_t[:], in_=ones_in[:, :])
            w42_t = pp.tile([P, P], bf16)
            nc_.sync.dma_start(out=w42_t[:], in_=w4_2_in[:, :])
            w43_t = pp.tile([P, P], bf16)
            nc_.sync.dma_start(out=w43_t[:], in_=w4_3_in[:, :])
            wl_t = pp.tile([HID, 3], f32)
            nc_.sync.dma_start(out=wl_t[:], in_=wl_in[:, :])
            idb_t = pp.tile([P, P], bf16)
            nc_.sync.dma_start(out=idb_t[:], in_=idb_in[:, :])
            idf_t = pp.tile([P, P], f32)
            nc_.sync.dma_start(out=idf_t[:], in_=idf_in[:, :])
            pidx_t = pp.tile([P, 16], i32)
            nc_.sync.dma_start(out=pidx_t[:], in_=pidx_in[:, :])
            prm_t = pp.tile([P, 16 * HID], f32)
            for jj in range(16):
                nc_.gpsimd.indirect_dma_start(
                    out=prm_t[:, jj * HID:(jj + 1) * HID],
                    out_offset=None, in_=prm_in[:],
                    in_offset=bass.IndirectOffsetOnAxis(
                        ap=pidx_t[:, jj:jj + 1], axis=0))
            ocol_t = pp.tile([P, 1], bf16)
            nc_.vector.memset(ocol_t[:], 1.0)

            zrow = pp.tile([1, 2 * HID], f32)
            nc_.vector.memset(zrow[:], 0.0)
            zrow_b = pp.tile([1, HID], bf16)
            nc_.vector.memset(zrow_b[:], 0.0)
            nc_.sync.dma_start(out=tabA[TBL:TBL + 1, :], in_=zrow_b[:])
            nc_.sync.dma_start(out=tabB[TBL:TBL + 1, :], in_=zrow_b[:])
            nc_.sync.dma_start(out=xb1[SH:SH + 1, :], in_=zrow_b[:])
            nc_.sync.dma_start(out=cbd[GPC:GPC + 1, :], in_=zrow[:])

            def prm_row(r):
                return prm_t[:, r * HID:(r + 1) * HID]

            def as3(ap):
                return ap.rearrange("p (x f) -> p x f", f=HID)

            def stats_sums(bounce, tag, want_sq=True):
                """bounce [SH+1] bf16 -> (s1, s2) per-graph sums of y and
                y^2, [GPC, HID] f32 each.  One gather set (real columns
                only, padded columns zeroed by memset); y^2 squared in
                SBUF from the gathered values."""
                st = sp.tile([P, 2 * GPC * HID], bf16, tag="stbuf")
                nc_.vector.memset(st[:], 0.0)
                for jj in range(2 * GPC):
                    if stat_col_real[jj]:
                        nc_.gpsimd.indirect_dma_start(
                            out=st[:, jj * HID:(jj + 1) * HID],
                            out_offset=None, in_=bounce[:],
                            in_offset=bass.IndirectOffsetOnAxis(
                                ap=idxs_t[:, jj:jj + 1], axis=0))
                if want_sq:
                    stq = sp.tile([P, 2 * GPC * HID], bf16, tag="stq")
                    nc_.vector.tensor_tensor(out=stq[:], in0=st[:], in1=st[:],
                                             op=AX.mult)

                def half(src, blk):
                    srow = wp.tile([1, NST * 512], f32, tag="srow")
                    with tc.tile_pool(name=f"psS{tag}{blk}", bufs=2,
                                      space="PSUM") as psS:
                        for m in range(NST):
                            pss = psS.tile([1, 512], f32, space="PSUM",
                                           tag="sps")
                            nc_.tensor.matmul(
                                out=pss[:], lhsT=ocol_t[:],
                                rhs=src[:, m * 512:(m + 1) * 512],
                                start=True, stop=True)
                            dst = srow[:, m * 512:(m + 1) * 512]
                            if m % 2 == 0:
                                nc_.scalar.activation(out=dst, in_=pss[:],
                                                      func=ACT.Copy)
                            else:
                                nc_.vector.tensor_copy(out=dst, in_=pss[:])
                    nc_.sync.dma_start(
                        out=srd[blk * 2 * GPC:(blk + 1) * 2 * GPC, :],
                        in_=srow[:])
                    sw = wp.tile([GPC, 2 * HID], f32, tag=f"sw{blk}")
                    nc_.sync.dma_start(
                        out=sw[:],
                        in_=srd[blk * 2 * GPC:(blk + 1) * 2 * GPC, :]
                        .rearrange("(g two) f -> g (two f)", two=2))
                    s = wp.tile([GPC, HID], f32, tag=tag + f"s{blk}")
                    nc_.vector.tensor_tensor(out=s[:], in0=sw[:, 0:HID],
                                             in1=sw[:, HID:2 * HID],
                                             op=AX.add)
                    return s

                s1 = half(st, 0)
                s2 = half(stq, 1) if want_sq else None
                return s1, s2

            descs = plan["descs"]
            col_base = plan["col_base"]
            ones_off = plan["ones_off"]
            tables = [None, tabA, tabB]
            x_prev = {}
            zb_prev = None

            for L in range(3):
                agg = wp.tile([P, NXF], f32, tag="agg")
                nc_.vector.memset(agg[:], 0.0)
                # ---------- aggregation ----------
                gbufs = {}
                with tc.tile_pool(name=f"psA{L}", bufs=2, space="PSUM") as psA:
                    ps = None
                    fg_list = []
                    last_fg = -1

                    def flush(ps, fg_list):
                        sc = wp.tile([P, FG_BANKS * 512], f32, tag="scr")
                        nc_.scalar.activation(out=sc[:], in_=ps[:],
                                              func=ACT.Copy)
                        for dd in fg_list:
                            nc_.sync.dma_start(
                                out=agg[dd["aggR"]:dd["aggR"] + dd["npc"],
                                        dd["band"] * HID:
                                        (dd["band"] + dd["ncols"]) * HID],
                                in_=sc[dd["lane"]:dd["lane"] + dd["npc"],
                                       dd["bank"] * 512:
                                       dd["bank"] * 512 + dd["ncols"] * HID])

                    for m, d in enumerate(descs):
                        gcol = int(col_base[d["k"]] + d["a"])
                        ch = gcol // GCH
                        if ch not in gbufs:
                            gb = gp.tile([P, GCH * HID], bf16, tag="gbuf")
                            if L == 0:
                                nc_.sync.dma_start(
                                    out=gb[:],
                                    in_=mg[:, ch * GCH * HID:
                                           (ch + 1) * GCH * HID])
                            else:
                                for j in range(GCH):
                                    jc = ch * GCH + j
                                    if jc < CWP and msg_col_real[jc]:
                                        nc_.gpsimd.indirect_dma_start(
                                            out=gb[:, j * HID:(j + 1) * HID],
                                            out_offset=None,
                                            in_=tables[L][:],
                                            in_offset=bass.IndirectOffsetOnAxis(
                                                ap=idxm_t[:, jc:jc + 1],
                                                axis=0))
                            gbufs[ch] = gb
                        if d["fg"] != last_fg:
                            if ps is not None:
                                flush(ps, fg_list)
                            ps = psA.tile([P, FG_BANKS * 512], f32,
                                          space="PSUM", tag="aggps")
                            if SIM_INIT_PSUM:
                                nc_.vector.memset(ps[:], 0.0)
                            fg_list = []
                            last_fg = d["fg"]
                        loc = gcol - ch * GCH
                        off = ones_off[d["k"]]
                        nc_.tensor.matmul(
                            out=ps[d["lane"]:d["lane"] + d["npc"],
                                   d["bank"] * 512:
                                   d["bank"] * 512 + d["ncols"] * HID],
                            lhsT=ones_t[:, off:off + d["npc"]],
                            rhs=gbufs[ch][:, loc * HID:(loc + d["ncols"]) * HID],
                            start=True, stop=True)
                        fg_list.append(d)
                    flush(ps, fg_list)

                # ---------- self-loop term: own table rows, held locally ----
                if L == 0:
                    sl = gp.tile([P, GCH * HID], bf16, tag="gbuf")
                    nc_.sync.dma_start(out=sl[:, 0:NXF], in_=slf[:, :])
                    sl_ap = sl[:, 0:NXF]
                else:
                    sl_ap = zb_prev[:]
                nc_.vector.tensor_tensor(out=agg[:], in0=agg[:],
                                         in1=sl_ap, op=AX.add)

                # ---------- y = agg * dinv ----------
                nc_.vector.tensor_tensor(
                    out=as3(agg[:]), in0=as3(agg[:]),
                    in1=dinv_t[:][:, :, None].to_broadcast([P, NX, HID]),
                    op=AX.mult)

                # ---------- graph norm (fused single pass) ----------
                nc_.gpsimd.dma_start(   # cast f32->bf16 in flight
                    out=xb1[0:SH, :].rearrange("(p x) f -> p (x f)", p=P),
                    in_=agg[:])
                s1, s2 = stats_sums(xb1, f"st{L}")

                # graph-space math: x = y + b;  m = E[x]; e2 = E[x^2]
                my = wp.tile([GPC, HID], f32, tag="my")
                nc_.vector.tensor_scalar_mul(out=my[:], in0=s1[:],
                                             scalar1=invc_t[:, 0:1])
                mm = wp.tile([GPC, HID], f32, tag="mm")
                nc_.vector.tensor_tensor(out=mm[:], in0=my[:],
                                         in1=prm_row(0 + L), op=AX.add)
                e2 = wp.tile([GPC, HID], f32, tag="e2")
                nc_.vector.tensor_scalar(out=e2[:], in0=s2[:],
                                         scalar1=invc_t[:, 0:1], scalar2=EPS,
                                         op0=AX.mult, op1=AX.add)
                u = wp.tile([GPC, HID], f32, tag="u")
                nc_.vector.tensor_tensor(out=u[:], in0=my[:], in1=mm[:],
                                         op=AX.add)
                nc_.vector.tensor_tensor(out=u[:], in0=u[:],
                                         in1=prm_row(0 + L), op=AX.mult)
                nc_.vector.tensor_tensor(out=e2[:], in0=e2[:], in1=u[:],
                                         op=AX.add)
                msm = wp.tile([GPC, HID], f32, tag="msm")
                nc_.vector.tensor_tensor(out=msm[:], in0=mm[:],
                                         in1=prm_row(9 + L), op=AX.mult)
                r = wp.tile([GPC, HID], f32, tag="r")
                nc_.vector.tensor_tensor(out=r[:], in0=mm[:], in1=msm[:],
                                         op=AX.subtract)
                nc_.vector.tensor_tensor(out=r[:], in0=r[:], in1=mm[:],
                                         op=AX.add)
                nc_.vector.tensor_tensor(out=r[:], in0=r[:], in1=msm[:],
                                         op=AX.mult)
                var = wp.tile([GPC, HID], f32, tag="var")
                nc_.vector.tensor_tensor(out=var[:], in0=e2[:], in1=r[:],
                                         op=AX.subtract)
                rstd = wp.tile([GPC, HID], f32, tag="rstd")
                nc_.vector.reciprocal(out=rstd[:], in_=var[:])
                nc_.scalar.activation(out=rstd[:], in_=rstd[:], func=ACT.Sqrt)
                ab = wp.tile([GPC, 2 * HID], f32, tag="ab")
                nc_.vector.tensor_tensor(out=ab[:, 0:HID], in0=rstd[:],
                                         in1=prm_row(6 + L), op=AX.mult)
                d2 = wp.tile([GPC, HID], f32, tag="d2")
                nc_.vector.tensor_tensor(out=d2[:], in0=prm_row(0 + L),
                                         in1=msm[:], op=AX.subtract)
                nc_.vector.tensor_tensor(out=d2[:], in0=d2[:],
                                         in1=ab[:, 0:HID], op=AX.mult)
                nc_.vector.tensor_tensor(out=ab[:, HID:2 * HID], in0=d2[:],
                                         in1=prm_row(3 + L), op=AX.add)
                # broadcast AB to node slots (256B row per graph)
                nc_.sync.dma_start(out=cbd[0:GPC, :], in_=ab[:])
                outb = wp.tile([P, NX * 2 * HID], f32, tag="bcbuf")
                for x in range(NX):
                    nc_.gpsimd.indirect_dma_start(
                        out=outb[:, x * 2 * HID:(x + 1) * 2 * HID],
                        out_offset=None, in_=cbd[:],
                        in_offset=bass.IndirectOffsetOnAxis(
                            ap=idxb_t[:, x:x + 1], axis=0))
                obv = outb[:].rearrange("p (x tf) -> p x tf", tf=2 * HID)
                # fused: xl = relu(y*A + B (+ x_prev))
                nc_.vector.tensor_tensor(out=as3(agg[:]), in0=as3(agg[:]),
                                         in1=obv[:, :, 0:HID], op=AX.mult)
                nc_.vector.tensor_tensor(out=as3(agg[:]), in0=as3(agg[:]),
                                         in1=obv[:, :, HID:2 * HID],
                                         op=AX.add)
                xl = wp.tile([P, NXF], bf16, tag=f"x{L % 2}")
                if L > 0:
                    nc_.vector.tensor_copy(out=xl[:], in_=agg[:])
                    nc_.vector.tensor_tensor(out=xl[:], in0=xl[:],
                                             in1=x_prev[L - 1][:], op=AX.add)
                    nc_.scalar.activation(out=xl[:], in_=xl[:], func=ACT.Relu)
                else:
                    nc_.scalar.activation(out=xl[:], in_=agg[:], func=ACT.Relu)
                x_prev[L] = xl

                if L < 2:
                    # ---------- next table ----------
                    w4 = w42_t if L == 0 else w43_t
                    tab = tabA if L == 0 else tabB
                    yb = wp.tile([P, NXF], bf16, tag="yb")
                    nc_.vector.tensor_tensor(
                        out=as3(yb[:]), in0=as3(xl[:]),
                        in1=dinv_b[:][:, :, None].to_broadcast([P, NX, HID]),
                        op=AX.mult)
                    xt4 = wp.tile([P, NXF], bf16, tag="xt4")
                    with tc.tile_pool(name=f"psT{L}", bufs=2, space="PSUM") as psT:
                        for b in range(NB):
                            pst = psT.tile([P, P], bf16, space="PSUM", tag="tp")
                            nc_.tensor.transpose(out=pst[:],
                                                 in_=yb[:, b * P:(b + 1) * P],
                                                 identity=idb_t[:])
                            nc_.vector.tensor_copy(
                                out=xt4[:, b * P:(b + 1) * P], in_=pst[:])
                    zt4 = wp.tile([P, NXF], bf16, tag="zt4")
                    with tc.tile_pool(name=f"psW{L}", bufs=2, space="PSUM") as psW:
                        for q in range(NXF // 512):
                            psw = psW.tile([P, 512], f32, space="PSUM", tag="wp")
                            nc_.tensor.matmul(out=psw[:], lhsT=w4[:],
                                              rhs=xt4[:, q * 512:(q + 1) * 512],
                                              start=True, stop=True)
                            nc_.vector.tensor_copy(
                                out=zt4[:, q * 512:(q + 1) * 512], in_=psw[:])
                    zb = wp.tile([P, NXF], bf16, tag="zb")
                    with tc.tile_pool(name=f"psU{L}", bufs=2, space="PSUM") as psU:
                        for b in range(NB):
                            pst = psU.tile([P, P], bf16, space="PSUM", tag="tp2")
                            nc_.tensor.transpose(out=pst[:],
                                                 in_=zt4[:, b * P:(b + 1) * P],
                                                 identity=idb_t[:])
                            nc_.scalar.activation(
                                out=zb[:, b * P:(b + 1) * P], in_=pst[:],
                                func=ACT.Copy)
                    nc_.sync.dma_start(
                        out=stg[:, :].rearrange("(p x) f -> p (x f)", p=P),
                        in_=zb[:])
                    zb_prev = zb
                    nc_.gpsimd.collective_compute(
                        "AllGather", AX.bypass, replica_groups=RG,
                        ins=[stg[:, :]], outs=[tab[0:TBL, :]])
                else:
                    # ---------- pool + logits ----------
                    nc_.sync.dma_start(
                        out=xb1[0:SH, :].rearrange("(p x) f -> p (x f)", p=P),
                        in_=xl[:])
                    s3, _s3q = stats_sums(xb1, "stF", want_sq=False)
                    pooled = wp.tile([GPC, HID], f32, tag="pooled")
                    nc_.vector.tensor_scalar_mul(out=pooled[:], in0=s3[:],
                                                 scalar1=invc_t[:, 0:1])
                    with tc.tile_pool(name="psF", bufs=1, space="PSUM") as psF:
                        pstp = psF.tile([HID, P], f32, space="PSUM", tag="pt")
                        nc_.tensor.transpose(out=pstp[:], in_=pooled[:],
                                             identity=idf_t[:])
                        pooledT = wp.tile([HID, P], f32, tag="pooledT")
                        nc_.vector.tensor_copy(out=pooledT[:], in_=pstp[:])
                        psl = psF.tile([P, 3], f32, space="PSUM", tag="lg")
                        nc_.tensor.matmul(out=psl[:], lhsT=pooledT[:],
                                          rhs=wl_t[:], start=True, stop=True)
                        logits = wp.tile([GPC, 3], f32, tag="logits")
                        nc_.vector.tensor_tensor(
                            out=logits[:], in0=psl[:],
                            in1=prm_t[0:GPC, 12 * HID:12 * HID + 3], op=AX.add)
                    nc_.sync.dma_start(out=lg_in[:, :], in_=logits[:])
                    nc_.gpsimd.collective_compute(
                        "AllGather", AX.bypass, replica_groups=RG,
                        ins=[lg_in[:, :]], outs=[lg_out[:, :]])
                    ofin = wp.tile([P, N_GRAPHS * 3 // P], f32, tag="ofin")
                    nc_.sync.dma_start(
                        out=ofin[:],
                        in_=lg_out[:, :].rearrange("(p a) f -> p (a f)", p=P))
                    nc_.sync.dma_start(
                        out=out_t[:, :].rearrange("(p a) f -> p (a f)", p=P),
                        in_=ofin[:])

    nc_.finalize()
    return nc_


_CACHE = {}


def _get_plan_nc(edge_index, batch):
    key = (hash(np.asarray(edge_index)[:, ::997].tobytes()),
           hash(np.asarray(batch)[::97].tobytes()))
    if key not in _CACHE:
        plan = _preprocess_structure(edge_index, batch)
        nc_ = _build(plan)
        _CACHE[key] = (plan, nc_)
    return _CACHE[key]


def kernel(x, edge_index, batch, W1, b1, W2, b2, W3, b3,
           g1, be1, ms1, g2, be2, ms2, g3, be3, ms3, Wl, bl):
    plan, nc_ = _get_plan_nc(edge_index, batch)
    NX, SH, TBL, CWP = plan["NX"], plan["SH"], plan["TBL"], plan["CWP"]
    dinv = plan["dinv"]
    r_global = plan["r_global"]

    x = np.asarray(x, np.float32)
    t1_np = np.zeros((TBL + 1, HID), np.float32)
    t1_np[r_global] = (x @ np.asarray(W1, np.float32)) * dinv[:, None]
    t1_np = t1_np.astype(BF)

    prm_np = np.zeros((16, HID), np.float32)
    for i, v in enumerate([b1, b2, b3, be1, be2, be3, g1, g2, g3,
                           ms1, ms2, ms3]):
        prm_np[i] = np.asarray(v, np.float32)
    prm_np[12, :3] = np.asarray(bl, np.float32)

    def blockdiag(w):
        w4 = np.zeros((P, P), np.float32)
        for s in range(4):
            w4[s * HID:(s + 1) * HID, s * HID:(s + 1) * HID] = \
                np.asarray(w, np.float32)
        return w4.astype(BF)

    in_maps = []
    base = dict(
        ones=plan["ones_all"], prm=prm_np,
        pidx=np.tile(np.arange(16, dtype=np.int32), (P, 1)),
        w4_2=blockdiag(W2), w4_3=blockdiag(W3),
        wl=np.asarray(Wl, np.float32),
        idb=np.eye(P, dtype=BF), idf=np.eye(P, dtype=np.float32),
    )
    agg_p, agg_x = plan["agg_p"], plan["agg_x"]
    core_start = plan["core_start"]
    for c in range(NC):
        # layer-1 messages pre-gathered on host
        mg_c = t1_np[plan["idx_msgs"][c]].reshape(P, CWP * HID)
        # layer-1 self-loop rows in slot layout
        ids = np.arange(core_start[c], core_start[c + 1])
        slf_c = np.zeros((P, NX, HID), BF)
        slf_c[agg_p[ids], agg_x[ids]] = t1_np[plan["r_global"][ids]]
        in_maps.append(dict(
            base,
            mg=mg_c,
            slf=slf_c.reshape(P, NX * HID),
            idxm=plan["idx_msgs"][c],
            idxs=plan["idx_stats"][c],
            idxb=plan["idx_bc"][c],
            dinv=plan["dinv_agg"][c],
            invc=plan["invcnt_col"][c][:, None].astype(np.float32),
        ))
    res = run_bass_kernel_spmd(nc_, in_maps, list(range(NC)))
    if DEBUG:
        kernel._last_results = res.results
    return np.asarray(res.results[0]["out"], np.float32)
